# revision 7
# baseline (speedup 1.0000x reference)
"""Paged GQA decode attention (sparse_attention) on 8 Trainium2 NeuronCores.

Problem (fp32): B=16 decode sequences, HQ=32 query heads, HKV=8 KV heads (GQA G=4),
D=128, paged KV cache with page_size=1 (SLOTS=65552 slots), ragged kv_len in
[2048, 4096], int32 page table kv_indices [B, L=4096].

reference:
  1) k_cache[slot_mapping] = k ; v_cache[slot_mapping] = v     (scatter new token)
  2) kk = k_cache[kv_indices], vv = v_cache[kv_indices]        (paged gather)
  3) GQA softmax(q.kk/sqrt(D)) @ vv  ->  out [B, HQ*D]

Sharding: batch-sharded, 2 sequences per core; both caches are fed REPLICATED
(each core reads only its 2 sequences' pages from its own HBM copy).

v2: fp16 end-to-end on device. The caches are converted to fp16 on the host
(rel err ~8e-4 vs the 2e-2 gate), halving the gather traffic, and the K gather
uses the SWDGE transpose mode (16-bit corner turn) so K lands in SBUF already
transposed [d, head, token] - the PE transpose stage and its PSUM->SBUF copy
from v1 are gone entirely.

Device kernel (one SPMD program for all 8 cores):
  - KV page gather via InstDMAGatherAnt (512 rows of 2KB per instruction).
    Indices are int16, so each sequence's token list is split on the host into
    slot-range groups (<32768, <65536, top-16) plus an "aux" group for tokens
    whose slot was overwritten by slot_mapping (those read the fresh k/v from a
    tiny side tensor instead, which also makes the reference's scatter step
    unnecessary on device).
  - Block compute per 128 gathered tokens: QK^T matmul per kv head with the
    transposed-gathered kT stationary giving transposed scores [l, hq]
    (softmax reduction lands on the free dim of the PV matmul), exp on the
    scalar engine with a per-block host-computed bias column (-1e30 masks
    ragged / padded lanes; gather buffers are pre-zeroed once so stale lanes
    are always finite), then V-stationary PV matmul accumulating o^T [d, hq]
    in PSUM, plus a ones-stationary matmul accumulating the softmax
    denominator.
  - Final: reciprocal of denominators, broadcast multiply, DMA out [128, 64].

The block layout (number of blocks per range group) is chosen per call from
the actual group sizes (max over all 16 sequences), so one compiled program is
shared by all cores; per-sequence raggedness is handled with runtime count
registers and the exp bias mask. Compiled programs are cached per layout.
"""
import sys
if '/opt/trn_rl_repo' not in sys.path:
    sys.path.insert(0, '/opt/trn_rl_repo')

import numpy as np

import concourse.bass as bass
import concourse.mybir as mybir
from concourse import bacc
from concourse.tile import TileContext

# ---- problem constants (hardcoded per contract) ----
B, HQ, HKV, D, L = 16, 32, 8, 128, 4096
G = HQ // HKV                 # 4 query heads per kv head
SLOTS = B * (L + 1)           # 65552
SCALE = 0.08838834764831845
N_CORES = 8
SEQ_PER_CORE = B // N_CORES   # 2
ROW = HKV * D                 # 1024 elems = one cache row (all kv heads)
GSZ = 512                     # tokens per gather chunk (= 4 blocks)
BLK = 128                     # tokens per compute block
NEG_BIAS = -1.0e30

FP32 = mybir.dt.float32
FP16 = mybir.dt.float16
I32 = mybir.dt.int32
I16 = mybir.dt.int16

import os
ABLATE = os.environ.get('KERNEL_ABLATE', '')


# --------------------------------------------------------------------------
# program builder
# --------------------------------------------------------------------------

def build_program(nblks, reps=1):
    """nblks: tuple (nb_lo, nb_hi, nb_g2, nb_aux) block counts per group.
    Returns compiled Bacc program."""
    nb = list(nblks)
    NBLK = sum(nb)                       # compute blocks per sequence
    # chunks per group (each chunk = one dma_gather of up to GSZ tokens)
    nchunks = [(x * BLK + GSZ - 1) // GSZ for x in nb]
    CHUNKS = sum(nchunks)                # per (seq, cache-shared) count entries
    IDXC = CHUNKS * (GSZ // 16)          # idx cols per seq
    bias_cols = 2 * NBLK
    # global block j -> (global chunk index, sub-block within chunk)
    blockmap = []
    cbase = 0
    for gi in range(4):
        for bo in range(nb[gi]):
            blockmap.append((cbase + bo // (GSZ // BLK), bo % (GSZ // BLK)))
        cbase += nchunks[gi]

    nc = bacc.Bacc("TRN2", target_bir_lowering=False, debug=False,
                   num_devices=N_CORES)
    kc = nc.dram_tensor("kc", [SLOTS, ROW], FP16, kind="ExternalInput")
    vc = nc.dram_tensor("vc", [SLOTS, ROW], FP16, kind="ExternalInput")
    kaux = nc.dram_tensor("kaux", [16, ROW], FP16, kind="ExternalInput")
    vaux = nc.dram_tensor("vaux", [16, ROW], FP16, kind="ExternalInput")
    qT = nc.dram_tensor("qT", [128, 2 * HQ], FP16, kind="ExternalInput")
    idx16 = nc.dram_tensor("idx16", [128, 2 * IDXC], I16, kind="ExternalInput")
    biasd = nc.dram_tensor("biasd", [128, bias_cols], FP32, kind="ExternalInput")
    cnts = nc.dram_tensor("cnts", [1, 2 * CHUNKS], I32, kind="ExternalInput")
    out = nc.dram_tensor("o", [128, 2 * HQ], FP32, kind="ExternalOutput")

    # source APs per group: (base AP, aux?)
    def group_src(cache, aux_tensor, gi):
        if gi == 0:
            return cache[0:32768, :]
        if gi == 1:
            return cache[32768:65536, :]
        if gi == 2:
            return cache[65536:SLOTS, :]
        return aux_tensor[:, :]

    with TileContext(nc) as tc:
        with (
            tc.tile_pool(name="const", bufs=1) as cpool,
            tc.tile_pool(name="kg", bufs=3) as kpool,
            tc.tile_pool(name="vg", bufs=3) as vpool,
            tc.tile_pool(name="pt", bufs=3) as ptpool,
            tc.tile_pool(name="fin", bufs=1) as fpool,
            tc.tile_pool(name="ps_st", bufs=2, space="PSUM") as ps_st,
            tc.tile_pool(name="ps_o", bufs=2, space="PSUM") as ps_o,
        ):
            ones_t = cpool.tile([128, 1], FP16)
            nc.vector.memset(ones_t[:], 1.0)
            qT_t = cpool.tile([128, 2 * HQ], FP16)
            nc.sync.dma_start(out=qT_t[:], in_=qT[:, :])
            idx_t = cpool.tile([128, 2 * IDXC], I16)
            nc.sync.dma_start(out=idx_t[:], in_=idx16[:, :])
            bias_t = cpool.tile([128, bias_cols], FP32)
            nc.sync.dma_start(out=bias_t[:], in_=biasd[:, :])
            cnt_t = cpool.tile([1, 2 * CHUNKS], I32)
            nc.sync.dma_start(out=cnt_t[:], in_=cnts[:, :])

            # per-seq accumulator bank: cols [0:32] = o^T, row0 cols [32:64]
            # = denom
            acc_tiles = [None, None]

            # pre-zero the rotating gather slots (on DVE, interleaved so the
            # first chunk's two buffers are ready first) so stale SBUF NaNs
            # can't reach the first blocks' matmuls; all later stale data is
            # previous-chunk cache values, which are finite fp16.
            for i in range(3):
                t = kpool.tile([128, (GSZ // BLK) * ROW], FP16, tag="kg")
                nc.vector.memset(t[:], 0.0)
                t = vpool.tile([128, (GSZ // BLK) * ROW], FP16, tag="vg")
                nc.vector.memset(t[:], 0.0)

            for _rep in range(reps):
                for s in range(SEQ_PER_CORE):
                    acc_s = ps_o.tile([128, 2 * HQ], FP32, space="PSUM",
                                      tag="acc")
                    acc_tiles[s] = acc_s
                    # ---- gathers for this sequence ----
                    ktiles, vtiles = [], []
                    ci_global = 0
                    for gi in range(4):
                        ksrc = group_src(kc, kaux, gi)
                        vsrc = group_src(vc, vaux, gi)
                        for ci in range(nchunks[gi]):
                            col0 = (s * IDXC + ci_global * (GSZ // 16))
                            iap = idx_t[:, col0:col0 + GSZ // 16]
                            creg = nc.values_load(
                                cnt_t[0:1, s * CHUNKS + ci_global:
                                      s * CHUNKS + ci_global + 1],
                                engines=(mybir.EngineType.Pool,),
                                min_val=1, max_val=GSZ,
                                skip_runtime_bounds_check=True)
                            # K: SWDGE 16-bit corner-turn gather ->
                            # kt[d, head, token]
                            kt = kpool.tile([128, HKV * GSZ], FP16, tag="kg")
                            nc.gpsimd.dma_gather(
                                out_ap=kt[:].rearrange("p (e t) -> p e t",
                                                       t=GSZ),
                                in_ap=ksrc, idxs_ap=iap,
                                num_idxs=GSZ, num_idxs_reg=creg,
                                elem_size=ROW, transpose=True)
                            # V: row gather -> vt[token, (subblk, head, d)]
                            vt = vpool.tile([128, (GSZ // BLK) * ROW], FP16,
                                            tag="vg")
                            nc.gpsimd.dma_gather(
                                out_ap=vt[:].rearrange("p (j e) -> p j e",
                                                       e=ROW),
                                in_ap=vsrc, idxs_ap=iap,
                                num_idxs=GSZ, num_idxs_reg=creg,
                                elem_size=ROW)
                            ktiles.append(kt)
                            vtiles.append(vt)
                            ci_global += 1

                    # ---- compute blocks (software-pipelined emission:
                    # PE stream runs QK(j), PV(j-2) so the cross-engine exp
                    # result is ready a block early) ----
                    def stage_QK(j):
                        ch, jj = blockmap[j]
                        kt = ktiles[ch]
                        sT_ps = ps_st.tile([128, HQ], FP32, space="PSUM",
                                           tag="sT_ps")
                        for h in range(HKV):
                            nc.tensor.matmul(
                                sT_ps[:, h*G:(h+1)*G],
                                kt[:, h*GSZ + jj*BLK: h*GSZ + (jj+1)*BLK],
                                qT_t[:, s*HQ + h*G: s*HQ + (h+1)*G],
                                start=True, stop=True)
                        return sT_ps

                    def stage_exp(j, sT_ps):
                        pT = ptpool.tile([128, HQ], FP16, tag="pt")
                        nc.scalar.activation(
                            pT[:], sT_ps[:],
                            mybir.ActivationFunctionType.Exp,
                            bias=bias_t[:, s*NBLK + j: s*NBLK + j + 1],
                            scale=SCALE)
                        return pT

                    def stage_PV(j, pT):
                        ch, jj = blockmap[j]
                        vt = vtiles[ch]
                        # one accumulation group per seq bank: started by the
                        # h=0 PV (covers all 128 partitions), denom rides along
                        nc.tensor.matmul(
                            acc_s[:, 0:G],
                            vt[:, jj*ROW: jj*ROW + D],
                            pT[:, 0:G],
                            start=(j == 0), stop=False)
                        nc.tensor.matmul(
                            acc_s[0:1, HQ:2*HQ],
                            ones_t[:], pT[:],
                            start=False, stop=False)
                        for h in range(1, HKV):
                            nc.tensor.matmul(
                                acc_s[:, h*G:(h+1)*G],
                                vt[:, jj*ROW + h*D: jj*ROW + (h+1)*D],
                                pT[:, h*G:(h+1)*G],
                                start=False,
                                stop=(j == NBLK - 1 and h == HKV - 1))

                    NB_RUN = NBLK if ABLATE != 'dmaonly' else 0
                    sTs, pTs = {}, {}
                    for jj in range(NB_RUN + 2):
                        if jj < NB_RUN:
                            sTs[jj] = stage_QK(jj)
                        if 1 <= jj and jj - 1 < NB_RUN:
                            pTs[jj - 1] = stage_exp(jj - 1, sTs.pop(jj - 1))
                        if 2 <= jj and jj - 2 < NB_RUN:
                            stage_PV(jj - 2, pTs.pop(jj - 2))

            # ---- normalize and write out ----
            o_sb = fpool.tile([128, 2 * HQ], FP32)
            if ABLATE == 'dmaonly':
                # keep a data dependency on the last gather tiles
                nc.vector.tensor_copy(o_sb[:, 0:1], ktiles[-1][:, 0:1])
                nc.vector.tensor_copy(o_sb[:, 1:2], vtiles[-1][:, 0:1])
            for s in range(SEQ_PER_CORE if ABLATE != 'dmaonly' else 0):
                acc_s = acc_tiles[s]
                rec = fpool.tile([1, HQ], FP32, tag=f"rec{s}")
                nc.vector.reciprocal(rec[:], acc_s[0:1, HQ:2*HQ])
                recb = fpool.tile([128, HQ], FP32, tag=f"recb{s}")
                nc.gpsimd.partition_broadcast(recb[:], rec[:])
                nc.vector.tensor_tensor(
                    out=o_sb[:, s*HQ:(s+1)*HQ], in0=acc_s[:, 0:HQ], in1=recb[:],
                    op=mybir.AluOpType.mult)
            nc.sync.dma_start(out=out[:, :], in_=o_sb[:])

    nc.compile()
    return nc


# --------------------------------------------------------------------------
# host-side input prep
# --------------------------------------------------------------------------

def balance_assignment(kv_len):
    """Greedy LPT: assign 16 seqs to 8 cores (2 each) minimizing max core
    token load (the kernel is gather-DMA-bound, so core time ~ sum kv_len).
    Returns perm with perm[2*c + s] = original sequence index."""
    order = np.argsort(-np.asarray(kv_len), kind="stable")
    loads = [0] * N_CORES
    slots = [[] for _ in range(N_CORES)]
    for b in order:
        c = min((c for c in range(N_CORES) if len(slots[c]) < 2),
                key=lambda c: loads[c])
        slots[c].append(int(b))
        loads[c] += int(kv_len[b])
    return np.array([b for c in range(N_CORES) for b in slots[c]], np.int64)


def prep_inputs(q, k, v, k_cache, v_cache, slot_mapping, kv_indices, kv_len):
    """Returns (nblks, in_maps, perm) — per-core input dicts."""
    q = np.asarray(q); k = np.asarray(k); v = np.asarray(v)
    k_cache = np.asarray(k_cache); v_cache = np.asarray(v_cache)
    slot_mapping = np.asarray(slot_mapping)
    kv_indices = np.asarray(kv_indices); kv_len = np.asarray(kv_len)
    perm = balance_assignment(kv_len)

    lut = np.full(SLOTS, -1, np.int32)
    lut[slot_mapping] = np.arange(16, dtype=np.int32)   # last write wins

    # per-seq group token lists
    groups = []   # groups[b] = [lo, hi, g2, aux] int arrays (rebased)
    for b in range(B):
        val = kv_indices[b, :int(kv_len[b])]
        a = lut[val]
        aux = a[a >= 0].astype(np.int64)
        rest = val[a < 0].astype(np.int64)
        lo = rest[rest < 32768]
        hi = rest[(rest >= 32768) & (rest < 65536)] - 32768
        g2 = rest[rest >= 65536] - 65536
        groups.append([lo, hi, g2, aux])

    nb = [max(1, max((len(groups[b][gi]) + BLK - 1) // BLK for b in range(B)))
          for gi in range(4)]
    nblks = tuple(nb)
    nchunks = [(x * BLK + GSZ - 1) // GSZ for x in nb]
    CHUNKS = sum(nchunks)
    NBLK = sum(nb)
    IDXC = CHUNKS * (GSZ // 16)

    kc16 = np.ascontiguousarray(k_cache.reshape(SLOTS, ROW), np.float16)
    vc16 = np.ascontiguousarray(v_cache.reshape(SLOTS, ROW), np.float16)
    kaux = np.ascontiguousarray(k.reshape(16, ROW), np.float16)
    vaux = np.ascontiguousarray(v.reshape(16, ROW), np.float16)

    in_maps = []
    for c in range(N_CORES):
        idxa = np.full((2, IDXC * 16), -1, np.int16).reshape(2, CHUNKS, GSZ)
        cnt = np.ones((2, CHUNKS), np.int32)
        biasa = np.full((128, 2 * NBLK), NEG_BIAS, np.float32)
        for s in range(SEQ_PER_CORE):
            b = int(perm[2 * c + s])
            ci_g = 0
            blk0 = 0
            for gi in range(4):
                toks = groups[b][gi]
                n = len(toks)
                for ci in range(nchunks[gi]):
                    part = toks[ci * GSZ:(ci + 1) * GSZ]
                    if len(part) == 0:
                        idxa[s, ci_g, 0] = 0     # dummy valid index, count 1
                        cnt[s, ci_g] = 1
                    else:
                        idxa[s, ci_g, :len(part)] = part.astype(np.int16)
                        cnt[s, ci_g] = len(part)
                    ci_g += 1
                # bias: valid lanes 0, masked lanes NEG_BIAS
                for bo in range(nb[gi]):
                    valid = min(max(n - bo * BLK, 0), BLK)
                    if valid > 0:
                        biasa[:valid, s * NBLK + blk0 + bo] = 0.0
                blk0 += nb[gi]
        # wrap idx to [16, x] then replicate to 128 partitions
        idx16 = np.concatenate(
            [np.tile(idxa[s].reshape(CHUNKS, GSZ // 16, 16)
                     .transpose(0, 2, 1).reshape(CHUNKS * 16, GSZ // 16)
                     .reshape(CHUNKS, 16, GSZ // 16)
                     .transpose(1, 0, 2).reshape(16, IDXC), (8, 1))
             for s in range(2)], axis=1)
        qTc = np.ascontiguousarray(
            q[perm[2*c:2*c+2]].transpose(2, 0, 1).reshape(128, 2 * HQ),
            np.float16)
        in_maps.append({
            "kc": kc16,
            "vc": vc16,
            "kaux": kaux, "vaux": vaux,
            "qT": qTc,
            "idx16": idx16,
            "biasd": biasa,
            "cnts": cnt.reshape(1, 2 * CHUNKS),
        })
    return nblks, in_maps, perm


def assemble_out(res, perm):
    """res: per-core {'o': [128, 64]} dicts -> full [B, HQ*D] output."""
    out = np.empty((B, HQ * D), np.float32)
    for c in range(N_CORES):
        o = res[c]["o"]                      # [128, 64] = [d, s*32+hq]
        for s in range(SEQ_PER_CORE):
            out[int(perm[2*c + s])] = o[:, s*HQ:(s+1)*HQ].T.reshape(HQ * D)
    return out


# --------------------------------------------------------------------------
# PJRT runner (replicated caches ship once)
# --------------------------------------------------------------------------

REPLICATED = ("kc", "vc", "kaux", "vaux")


class BassRunner:
    def __init__(self, nc, n_cores, replicated=()):
        import jax
        from jax.sharding import Mesh, PartitionSpec, NamedSharding
        from jax.experimental.shard_map import shard_map
        from concourse.bass2jax import (_bass_exec_p, partition_id_tensor,
                                        install_neuronx_cc_hook)
        install_neuronx_cc_hook()
        self.jax = jax
        self.nc = nc
        self.n_cores = n_cores
        self.replicated = set(replicated)
        in_names, out_names, out_avals, zero_outs = [], [], [], []
        partition_name = (nc.partition_id_tensor.name
                          if nc.partition_id_tensor else None)
        for alloc in nc.m.functions[0].allocations:
            if not isinstance(alloc, mybir.MemoryLocationSet):
                continue
            name = alloc.memorylocations[0].name
            if alloc.kind == "ExternalInput":
                if name != partition_name:
                    in_names.append(name)
            elif alloc.kind == "ExternalOutput":
                shape = tuple(alloc.tensor_shape)
                dtype = mybir.dt.np(alloc.dtype)
                out_names.append(name)
                out_avals.append(jax.core.ShapedArray(shape, dtype))
                zero_outs.append(np.zeros(shape, dtype))
        self.in_names, self.out_names = in_names, out_names
        self.out_avals, self.zero_outs = out_avals, zero_outs
        n_params = len(in_names)
        all_in_names = list(in_names) + list(out_names)
        if partition_name is not None:
            all_in_names.append(partition_name)

        def _body(*args):
            operands = list(args)
            if partition_name is not None:
                operands.append(partition_id_tensor())
            outs = _bass_exec_p.bind(
                *operands, out_avals=tuple(out_avals),
                in_names=tuple(all_in_names), out_names=tuple(out_names),
                lowering_input_output_aliases=(),
                sim_require_finite=True, sim_require_nnan=True, nc=nc)
            return tuple(outs)

        devices = jax.devices()[:n_cores]
        self.mesh = Mesh(np.asarray(devices), ("core",))
        self.sharding = NamedSharding(self.mesh, PartitionSpec("core"))
        self.rep_sharding = NamedSharding(self.mesh, PartitionSpec())
        in_specs = tuple(
            PartitionSpec() if n in self.replicated else PartitionSpec("core")
            for n in in_names) + (PartitionSpec("core"),) * len(out_names)
        out_specs = (PartitionSpec("core"),) * len(out_names)
        self.fn = jax.jit(
            shard_map(_body, mesh=self.mesh, in_specs=in_specs,
                      out_specs=out_specs, check_rep=False),
            keep_unused=True)

    def put_inputs(self, in_maps):
        args = []
        for name in self.in_names:
            if name in self.replicated:
                args.append(self.jax.device_put(np.asarray(in_maps[0][name]),
                                                self.rep_sharding))
            else:
                concat = np.concatenate(
                    [np.asarray(m[name]) for m in in_maps], axis=0)
                args.append(self.jax.device_put(concat, self.sharding))
        for z in self.zero_outs:
            zz = np.zeros((self.n_cores * z.shape[0], *z.shape[1:]), z.dtype)
            args.append(self.jax.device_put(zz, self.sharding))
        return args

    def run(self, args):
        outs = self.fn(*args)
        self.jax.block_until_ready(outs)
        return outs

    def results(self, outs):
        return [
            {name: np.asarray(outs[i]).reshape(
                self.n_cores, *self.out_avals[i].shape)[c]
             for i, name in enumerate(self.out_names)}
            for c in range(self.n_cores)
        ]


_RUNNER_CACHE = {}


def get_runner(nblks, reps=1):
    key = (nblks, reps, ABLATE)
    if key not in _RUNNER_CACHE:
        nc = build_program(nblks, reps=reps)
        _RUNNER_CACHE[key] = BassRunner(nc, N_CORES, replicated=REPLICATED)
    return _RUNNER_CACHE[key]


def kernel(**inputs) -> np.ndarray:
    nblks, in_maps, perm = prep_inputs(**inputs)
    runner = get_runner(nblks)
    args = runner.put_inputs(in_maps)
    res = runner.results(runner.run(args))
    return assemble_out(res, perm)


# revision 12
# speedup vs baseline: 3.5265x; 3.5265x over previous
"""Paged GQA decode attention (sparse_attention) on 8 Trainium2 NeuronCores.

Problem (fp32): B=16 decode sequences, HQ=32 query heads, HKV=8 KV heads (GQA G=4),
D=128, paged KV cache with page_size=1 (SLOTS=65552 slots), ragged kv_len in
[2048, 4096], int32 page table kv_indices [B, L=4096].

reference:
  1) k_cache[slot_mapping] = k ; v_cache[slot_mapping] = v     (scatter new token)
  2) kk = k_cache[kv_indices], vv = v_cache[kv_indices]        (paged gather)
  3) GQA softmax(q.kk/sqrt(D)) @ vv  ->  out [B, HQ*D]

Sharding: batch-sharded, 2 sequences per core (host-balanced by kv_len); the
cache is fed REPLICATED (each core reads only its 2 sequences' pages from its
own HBM copy).

v3: fp16 + a single combined KV row per slot. Measurements showed the paged
gather is per-descriptor-bound (~190ns/row regardless of 2KB vs 4KB row), so
the host interleaves k_cache and v_cache into one [SLOTS, 2048] fp16 tensor
(4KB rows) and each chunk needs ONE gather instruction instead of two -
halving the descriptor count vs the f32 baseline while also halving bytes.

Device kernel (one SPMD program for all 8 cores):
  - KV page gather via InstDMAGatherAnt (512 rows of 4KB per instruction).
    Indices are int16, so each sequence's token list is split on the host into
    slot-range groups (<32768, <65536, top-16) plus an "aux" group for tokens
    whose slot was overwritten by slot_mapping (those read the fresh k/v from a
    tiny side tensor instead, which also makes the reference's scatter step
    unnecessary on device).
  - Block compute per 128 gathered tokens: PE transpose of the K half -> kT
    (fp16, 1 cycle/row), QK^T matmul with kT stationary giving transposed
    scores [l, hq] (softmax reduction lands on the free dim of the PV matmul),
    exp on the scalar engine with a per-block host-computed bias column
    (-1e30 masks ragged / padded lanes; gather buffers are pre-zeroed once so
    stale lanes are always finite fp16), then V-stationary PV matmul
    accumulating o^T [d, hq] in PSUM, plus a ones-stationary matmul
    accumulating the softmax denominator.
  - Final: reciprocal of denominators, broadcast multiply, DMA out [128, 64].

The block layout (number of blocks per range group) is chosen per call from
the actual group sizes (max over all 16 sequences), so one compiled program is
shared by all cores; per-sequence raggedness is handled with runtime count
registers and the exp bias mask. Compiled programs are cached per layout.
"""
import sys
if '/opt/trn_rl_repo' not in sys.path:
    sys.path.insert(0, '/opt/trn_rl_repo')

import numpy as np

import concourse.bass as bass
import concourse.mybir as mybir
from concourse import bacc
from concourse.tile import TileContext
from concourse.masks import make_identity

# ---- problem constants (hardcoded per contract) ----
B, HQ, HKV, D, L = 16, 32, 8, 128, 4096
G = HQ // HKV                 # 4 query heads per kv head
SLOTS = B * (L + 1)           # 65552
SCALE = 0.08838834764831845
N_CORES = 8
SEQ_PER_CORE = B // N_CORES   # 2
ROW = HKV * D                 # 1024 elems = one k (or v) row
KVROW = 2 * ROW               # combined k|v row, 2048 fp16 elems = 4KB
BLK = 128                     # tokens per compute block
NEG_BIAS = -1.0e30

FP32 = mybir.dt.float32
FP16 = mybir.dt.float16
I32 = mybir.dt.int32
I16 = mybir.dt.int16

import os
ABLATE = os.environ.get('KERNEL_ABLATE', '')
GSZ = int(os.environ.get('KERNEL_GSZ', '512'))   # tokens per gather chunk
NQ = int(os.environ.get('KERNEL_NQ', '1'))       # SWDGE queues to spread over


# --------------------------------------------------------------------------
# program builder
# --------------------------------------------------------------------------

def build_program(nblks, reps=1):
    """nblks: tuple (nb_lo, nb_hi, nb_g2, nb_aux) block counts per group.
    Returns compiled Bacc program."""
    nb = list(nblks)
    NBLK = sum(nb)                       # compute blocks per sequence
    # chunks per group (each chunk = one dma_gather of up to GSZ tokens)
    nchunks = [(x * BLK + GSZ - 1) // GSZ for x in nb]
    CHUNKS = sum(nchunks)                # per (seq) count entries
    IDXC = CHUNKS * (GSZ // 16)          # idx cols per seq
    bias_cols = 2 * NBLK
    # global block j -> (global chunk index, sub-block within chunk)
    blockmap = []
    cbase = 0
    for gi in range(4):
        for bo in range(nb[gi]):
            blockmap.append((cbase + bo // (GSZ // BLK), bo % (GSZ // BLK)))
        cbase += nchunks[gi]

    nc = bacc.Bacc("TRN2", target_bir_lowering=False, debug=False,
                   num_devices=N_CORES, num_swdge_queues=NQ)
    kvc = nc.dram_tensor("kvc", [SLOTS, KVROW], FP16, kind="ExternalInput")
    kvaux = nc.dram_tensor("kvaux", [16, KVROW], FP16, kind="ExternalInput")
    qT = nc.dram_tensor("qT", [128, 2 * HQ], FP16, kind="ExternalInput")
    idx16 = nc.dram_tensor("idx16", [128, 2 * IDXC], I16, kind="ExternalInput")
    biasd = nc.dram_tensor("biasd", [128, bias_cols], FP32, kind="ExternalInput")
    cnts = nc.dram_tensor("cnts", [1, 2 * CHUNKS], I32, kind="ExternalInput")
    out = nc.dram_tensor("o", [128, 2 * HQ], FP32, kind="ExternalOutput")

    def group_src(gi):
        if gi == 0:
            return kvc[0:32768, :]
        if gi == 1:
            return kvc[32768:65536, :]
        if gi == 2:
            return kvc[65536:SLOTS, :]
        return kvaux[:, :]

    with TileContext(nc) as tc:
        with (
            tc.tile_pool(name="const", bufs=1) as cpool,
            tc.tile_pool(name="kv", bufs=3) as kvpool,
            tc.tile_pool(name="kt", bufs=2) as ktpool,
            tc.tile_pool(name="pt", bufs=3) as ptpool,
            tc.tile_pool(name="fin", bufs=1) as fpool,
            tc.tile_pool(name="ps_kt", bufs=2, space="PSUM") as ps_kt,
            tc.tile_pool(name="ps_st", bufs=2, space="PSUM") as ps_st,
            tc.tile_pool(name="ps_o", bufs=2, space="PSUM") as ps_o,
        ):
            ident = cpool.tile([128, 128], FP16)
            make_identity(nc, ident[:])
            ones_t = cpool.tile([128, 1], FP16)
            nc.vector.memset(ones_t[:], 1.0)
            qT_t = cpool.tile([128, 2 * HQ], FP16)
            nc.sync.dma_start(out=qT_t[:], in_=qT[:, :])
            idx_t = cpool.tile([128, 2 * IDXC], I16)
            nc.sync.dma_start(out=idx_t[:], in_=idx16[:, :])
            bias_t = cpool.tile([128, bias_cols], FP32)
            nc.sync.dma_start(out=bias_t[:], in_=biasd[:, :])
            cnt_t = cpool.tile([1, 2 * CHUNKS], I32)
            nc.sync.dma_start(out=cnt_t[:], in_=cnts[:, :])

            # per-seq accumulator bank: cols [0:32] = o^T, row0 cols [32:64]
            # = denom
            acc_tiles = [None, None]

            # pre-zero the rotating gather slots so stale SBUF NaNs can't
            # reach the first blocks' matmuls; all later stale data is
            # previous-chunk cache values, which are finite fp16.
            for i in range(3):
                t = kvpool.tile([128, (GSZ // BLK) * KVROW], FP16, tag="kv")
                eng = (nc.vector, nc.gpsimd)[i % 2]
                eng.memset(t[:], 0.0)

            for _rep in range(reps):
                for s in range(SEQ_PER_CORE):
                    acc_s = ps_o.tile([128, 2 * HQ], FP32, space="PSUM",
                                      tag="acc")
                    acc_tiles[s] = acc_s
                    # ---- gathers for this sequence ----
                    kvtiles = []
                    ci_global = 0
                    for gi in range(4):
                        src = group_src(gi)
                        for ci in range(nchunks[gi]):
                            col0 = (s * IDXC + ci_global * (GSZ // 16))
                            iap = idx_t[:, col0:col0 + GSZ // 16]
                            creg = nc.values_load(
                                cnt_t[0:1, s * CHUNKS + ci_global:
                                      s * CHUNKS + ci_global + 1],
                                engines=(mybir.EngineType.Pool,),
                                min_val=1, max_val=GSZ,
                                skip_runtime_bounds_check=True)
                            kv = kvpool.tile([128, (GSZ // BLK) * KVROW], FP16,
                                             tag="kv")
                            nc.gpsimd.dma_gather(
                                out_ap=kv[:].rearrange("p (j e) -> p j e",
                                                       e=KVROW),
                                in_ap=src, idxs_ap=iap,
                                num_idxs=GSZ, num_idxs_reg=creg,
                                elem_size=KVROW,
                                queue_num=ci_global % NQ)
                            kvtiles.append(kv)
                            ci_global += 1

                    # ---- compute blocks (software-pipelined emission:
                    # PE stream runs T(j), QK(j-1), PV(j-2) so cross-engine
                    # results (kT copy, exp) are ready a block early) ----
                    def stage_T(j):
                        ch, jj = blockmap[j]
                        kv = kvtiles[ch]
                        kT_ps = ps_kt.tile([128, ROW], FP16, space="PSUM",
                                           tag="kT_ps")
                        for h in range(HKV):
                            nc.tensor.transpose(
                                kT_ps[:, h*D:(h+1)*D],
                                kv[:, jj*KVROW + h*D: jj*KVROW + (h+1)*D],
                                ident[:])
                        kT_sb = ktpool.tile([128, ROW], FP16, tag="kt")
                        if j % 2 == 0:
                            nc.scalar.copy(kT_sb[:], kT_ps[:])
                        else:
                            nc.vector.tensor_copy(kT_sb[:], kT_ps[:])
                        return kT_sb

                    def stage_Q(j, kT_sb):
                        sT_ps = ps_st.tile([128, HQ], FP32, space="PSUM",
                                           tag="sT_ps")
                        for h in range(HKV):
                            nc.tensor.matmul(
                                sT_ps[:, h*G:(h+1)*G],
                                kT_sb[:, h*D:(h+1)*D],
                                qT_t[:, s*HQ + h*G: s*HQ + (h+1)*G],
                                start=True, stop=True)
                        pT = ptpool.tile([128, HQ], FP16, tag="pt")
                        nc.scalar.activation(
                            pT[:], sT_ps[:],
                            mybir.ActivationFunctionType.Exp,
                            bias=bias_t[:, s*NBLK + j: s*NBLK + j + 1],
                            scale=SCALE)
                        return pT

                    def stage_P(j, pT):
                        ch, jj = blockmap[j]
                        kv = kvtiles[ch]
                        voff = jj * KVROW + ROW
                        # one accumulation group per seq bank: started by the
                        # h=0 PV (covers all 128 partitions), denom rides along
                        nc.tensor.matmul(
                            acc_s[:, 0:G],
                            kv[:, voff: voff + D],
                            pT[:, 0:G],
                            start=(j == 0), stop=False)
                        nc.tensor.matmul(
                            acc_s[0:1, HQ:2*HQ],
                            ones_t[:], pT[:],
                            start=False, stop=False)
                        for h in range(1, HKV):
                            nc.tensor.matmul(
                                acc_s[:, h*G:(h+1)*G],
                                kv[:, voff + h*D: voff + (h+1)*D],
                                pT[:, h*G:(h+1)*G],
                                start=False,
                                stop=(j == NBLK - 1 and h == HKV - 1))

                    NB_RUN = NBLK if ABLATE != 'dmaonly' else 0
                    kTs, pTs = {}, {}
                    for jj in range(NB_RUN + 2):
                        if jj < NB_RUN:
                            kTs[jj] = stage_T(jj)
                        if 1 <= jj and jj - 1 < NB_RUN:
                            pTs[jj - 1] = stage_Q(jj - 1, kTs.pop(jj - 1))
                        if 2 <= jj and jj - 2 < NB_RUN:
                            stage_P(jj - 2, pTs.pop(jj - 2))

            # ---- normalize and write out ----
            o_sb = fpool.tile([128, 2 * HQ], FP32)
            if ABLATE == 'dmaonly':
                # keep a data dependency on the last gather tiles
                nc.vector.tensor_copy(o_sb[:, 0:1], kvtiles[-1][:, 0:1])
                nc.vector.tensor_copy(o_sb[:, 1:2], kvtiles[-2][:, 0:1])
            for s in range(SEQ_PER_CORE if ABLATE != 'dmaonly' else 0):
                acc_s = acc_tiles[s]
                rec = fpool.tile([1, HQ], FP32, tag=f"rec{s}")
                nc.vector.reciprocal(rec[:], acc_s[0:1, HQ:2*HQ])
                recb = fpool.tile([128, HQ], FP32, tag=f"recb{s}")
                nc.gpsimd.partition_broadcast(recb[:], rec[:])
                nc.vector.tensor_tensor(
                    out=o_sb[:, s*HQ:(s+1)*HQ], in0=acc_s[:, 0:HQ], in1=recb[:],
                    op=mybir.AluOpType.mult)
            nc.sync.dma_start(out=out[:, :], in_=o_sb[:])

    nc.compile()
    return nc


# --------------------------------------------------------------------------
# host-side input prep
# --------------------------------------------------------------------------

def balance_assignment(kv_len):
    """Greedy LPT: assign 16 seqs to 8 cores (2 each) minimizing max core
    token load (the kernel is gather-DMA-bound, so core time ~ sum kv_len).
    Returns perm with perm[2*c + s] = original sequence index."""
    order = np.argsort(-np.asarray(kv_len), kind="stable")
    loads = [0] * N_CORES
    slots = [[] for _ in range(N_CORES)]
    for b in order:
        c = min((c for c in range(N_CORES) if len(slots[c]) < 2),
                key=lambda c: loads[c])
        slots[c].append(int(b))
        loads[c] += int(kv_len[b])
    return np.array([b for c in range(N_CORES) for b in slots[c]], np.int64)


def prep_inputs(q, k, v, k_cache, v_cache, slot_mapping, kv_indices, kv_len):
    """Returns (nblks, in_maps, perm) — per-core input dicts."""
    q = np.asarray(q); k = np.asarray(k); v = np.asarray(v)
    k_cache = np.asarray(k_cache); v_cache = np.asarray(v_cache)
    slot_mapping = np.asarray(slot_mapping)
    kv_indices = np.asarray(kv_indices); kv_len = np.asarray(kv_len)
    perm = balance_assignment(kv_len)

    lut = np.full(SLOTS, -1, np.int32)
    lut[slot_mapping] = np.arange(16, dtype=np.int32)   # last write wins

    # per-seq group token lists
    groups = []   # groups[b] = [lo, hi, g2, aux] int arrays (rebased)
    for b in range(B):
        val = kv_indices[b, :int(kv_len[b])]
        a = lut[val]
        aux = a[a >= 0].astype(np.int64)
        rest = val[a < 0].astype(np.int64)
        lo = rest[rest < 32768]
        hi = rest[(rest >= 32768) & (rest < 65536)] - 32768
        g2 = rest[rest >= 65536] - 65536
        groups.append([lo, hi, g2, aux])

    nb = [max(1, max((len(groups[b][gi]) + BLK - 1) // BLK for b in range(B)))
          for gi in range(4)]
    nblks = tuple(nb)
    nchunks = [(x * BLK + GSZ - 1) // GSZ for x in nb]
    CHUNKS = sum(nchunks)
    NBLK = sum(nb)
    IDXC = CHUNKS * (GSZ // 16)

    # combined k|v rows, fp16
    kv16 = np.empty((SLOTS, KVROW), np.float16)
    kv16[:, :ROW] = k_cache.reshape(SLOTS, ROW)
    kv16[:, ROW:] = v_cache.reshape(SLOTS, ROW)
    kvaux = np.empty((16, KVROW), np.float16)
    kvaux[:, :ROW] = k.reshape(16, ROW)
    kvaux[:, ROW:] = v.reshape(16, ROW)

    in_maps = []
    for c in range(N_CORES):
        idxa = np.full((2, IDXC * 16), -1, np.int16).reshape(2, CHUNKS, GSZ)
        cnt = np.ones((2, CHUNKS), np.int32)
        biasa = np.full((128, 2 * NBLK), NEG_BIAS, np.float32)
        for s in range(SEQ_PER_CORE):
            b = int(perm[2 * c + s])
            ci_g = 0
            blk0 = 0
            for gi in range(4):
                toks = groups[b][gi]
                n = len(toks)
                for ci in range(nchunks[gi]):
                    part = toks[ci * GSZ:(ci + 1) * GSZ]
                    if len(part) == 0:
                        idxa[s, ci_g, 0] = 0     # dummy valid index, count 1
                        cnt[s, ci_g] = 1
                    else:
                        idxa[s, ci_g, :len(part)] = part.astype(np.int16)
                        cnt[s, ci_g] = len(part)
                    ci_g += 1
                # bias: valid lanes 0, masked lanes NEG_BIAS
                for bo in range(nb[gi]):
                    valid = min(max(n - bo * BLK, 0), BLK)
                    if valid > 0:
                        biasa[:valid, s * NBLK + blk0 + bo] = 0.0
                blk0 += nb[gi]
        # wrap idx to [16, x] then replicate to 128 partitions
        idx16 = np.concatenate(
            [np.tile(idxa[s].reshape(CHUNKS, GSZ // 16, 16)
                     .transpose(0, 2, 1).reshape(CHUNKS * 16, GSZ // 16)
                     .reshape(CHUNKS, 16, GSZ // 16)
                     .transpose(1, 0, 2).reshape(16, IDXC), (8, 1))
             for s in range(2)], axis=1)
        qTc = np.ascontiguousarray(
            q[perm[2*c:2*c+2]].transpose(2, 0, 1).reshape(128, 2 * HQ),
            np.float16)
        in_maps.append({
            "kvc": kv16,
            "kvaux": kvaux,
            "qT": qTc,
            "idx16": idx16,
            "biasd": biasa,
            "cnts": cnt.reshape(1, 2 * CHUNKS),
        })
    return nblks, in_maps, perm


def assemble_out(res, perm):
    """res: per-core {'o': [128, 64]} dicts -> full [B, HQ*D] output."""
    out = np.empty((B, HQ * D), np.float32)
    for c in range(N_CORES):
        o = res[c]["o"]                      # [128, 64] = [d, s*32+hq]
        for s in range(SEQ_PER_CORE):
            out[int(perm[2*c + s])] = o[:, s*HQ:(s+1)*HQ].T.reshape(HQ * D)
    return out


# --------------------------------------------------------------------------
# PJRT runner (replicated caches ship once)
# --------------------------------------------------------------------------

REPLICATED = ("kvc", "kvaux")


class BassRunner:
    def __init__(self, nc, n_cores, replicated=()):
        import jax
        from jax.sharding import Mesh, PartitionSpec, NamedSharding
        from jax.experimental.shard_map import shard_map
        from concourse.bass2jax import (_bass_exec_p, partition_id_tensor,
                                        install_neuronx_cc_hook)
        install_neuronx_cc_hook()
        self.jax = jax
        self.nc = nc
        self.n_cores = n_cores
        self.replicated = set(replicated)
        in_names, out_names, out_avals, zero_outs = [], [], [], []
        partition_name = (nc.partition_id_tensor.name
                          if nc.partition_id_tensor else None)
        for alloc in nc.m.functions[0].allocations:
            if not isinstance(alloc, mybir.MemoryLocationSet):
                continue
            name = alloc.memorylocations[0].name
            if alloc.kind == "ExternalInput":
                if name != partition_name:
                    in_names.append(name)
            elif alloc.kind == "ExternalOutput":
                shape = tuple(alloc.tensor_shape)
                dtype = mybir.dt.np(alloc.dtype)
                out_names.append(name)
                out_avals.append(jax.core.ShapedArray(shape, dtype))
                zero_outs.append(np.zeros(shape, dtype))
        self.in_names, self.out_names = in_names, out_names
        self.out_avals, self.zero_outs = out_avals, zero_outs
        n_params = len(in_names)
        all_in_names = list(in_names) + list(out_names)
        if partition_name is not None:
            all_in_names.append(partition_name)

        def _body(*args):
            operands = list(args)
            if partition_name is not None:
                operands.append(partition_id_tensor())
            outs = _bass_exec_p.bind(
                *operands, out_avals=tuple(out_avals),
                in_names=tuple(all_in_names), out_names=tuple(out_names),
                lowering_input_output_aliases=(),
                sim_require_finite=True, sim_require_nnan=True, nc=nc)
            return tuple(outs)

        devices = jax.devices()[:n_cores]
        self.mesh = Mesh(np.asarray(devices), ("core",))
        self.sharding = NamedSharding(self.mesh, PartitionSpec("core"))
        self.rep_sharding = NamedSharding(self.mesh, PartitionSpec())
        in_specs = tuple(
            PartitionSpec() if n in self.replicated else PartitionSpec("core")
            for n in in_names) + (PartitionSpec("core"),) * len(out_names)
        out_specs = (PartitionSpec("core"),) * len(out_names)
        self.fn = jax.jit(
            shard_map(_body, mesh=self.mesh, in_specs=in_specs,
                      out_specs=out_specs, check_rep=False),
            keep_unused=True)

    def put_inputs(self, in_maps):
        args = []
        for name in self.in_names:
            if name in self.replicated:
                args.append(self.jax.device_put(np.asarray(in_maps[0][name]),
                                                self.rep_sharding))
            else:
                concat = np.concatenate(
                    [np.asarray(m[name]) for m in in_maps], axis=0)
                args.append(self.jax.device_put(concat, self.sharding))
        for z in self.zero_outs:
            zz = np.zeros((self.n_cores * z.shape[0], *z.shape[1:]), z.dtype)
            args.append(self.jax.device_put(zz, self.sharding))
        return args

    def run(self, args):
        outs = self.fn(*args)
        self.jax.block_until_ready(outs)
        return outs

    def results(self, outs):
        return [
            {name: np.asarray(outs[i]).reshape(
                self.n_cores, *self.out_avals[i].shape)[c]
             for i, name in enumerate(self.out_names)}
            for c in range(self.n_cores)
        ]


_RUNNER_CACHE = {}


def get_runner(nblks, reps=1):
    key = (nblks, reps, ABLATE, GSZ, NQ)
    if key not in _RUNNER_CACHE:
        nc = build_program(nblks, reps=reps)
        _RUNNER_CACHE[key] = BassRunner(nc, N_CORES, replicated=REPLICATED)
    return _RUNNER_CACHE[key]


def kernel(**inputs) -> np.ndarray:
    nblks, in_maps, perm = prep_inputs(**inputs)
    runner = get_runner(nblks)
    args = runner.put_inputs(in_maps)
    res = runner.results(runner.run(args))
    return assemble_out(res, perm)


# revision 17
# speedup vs baseline: 4.5827x; 1.2995x over previous
"""Paged GQA decode attention (sparse_attention) on 8 Trainium2 NeuronCores.

Problem (fp32): B=16 decode sequences, HQ=32 query heads, HKV=8 KV heads (GQA G=4),
D=128, paged KV cache with page_size=1 (SLOTS=65552 slots), ragged kv_len in
[2048, 4096], int32 page table kv_indices [B, L=4096].

reference:
  1) k_cache[slot_mapping] = k ; v_cache[slot_mapping] = v     (scatter new token)
  2) kk = k_cache[kv_indices], vv = v_cache[kv_indices]        (paged gather)
  3) GQA softmax(q.kk/sqrt(D)) @ vv  ->  out [B, HQ*D]

Sharding: batch-sharded, 2 sequences per core (host-balanced by kv_len); the
cache is fed REPLICATED (each core reads only its 2 sequences' pages from its
own HBM copy).

v3: fp16 + a single combined KV row per slot. Measurements showed the paged
gather is per-descriptor-bound (~190ns/row regardless of 2KB vs 4KB row), so
the host interleaves k_cache and v_cache into one [SLOTS, 2048] fp16 tensor
(4KB rows) and each chunk needs ONE gather instruction instead of two -
halving the descriptor count vs the f32 baseline while also halving bytes.

Device kernel (one SPMD program for all 8 cores):
  - KV page gather via InstDMAGatherAnt (512 rows of 4KB per instruction).
    Indices are int16, so each sequence's token list is split on the host into
    slot-range groups (<32768, <65536, top-16) plus an "aux" group for tokens
    whose slot was overwritten by slot_mapping (those read the fresh k/v from a
    tiny side tensor instead, which also makes the reference's scatter step
    unnecessary on device).
  - Block compute per 128 gathered tokens: PE transpose of the K half -> kT
    (fp16, 1 cycle/row), QK^T matmul with kT stationary giving transposed
    scores [l, hq] (softmax reduction lands on the free dim of the PV matmul),
    exp on the scalar engine with a per-block host-computed bias column
    (-1e30 masks ragged / padded lanes; gather buffers are pre-zeroed once so
    stale lanes are always finite fp16), then V-stationary PV matmul
    accumulating o^T [d, hq] in PSUM, plus a ones-stationary matmul
    accumulating the softmax denominator.
  - Final: reciprocal of denominators, broadcast multiply, DMA out [128, 64].

The block layout (number of blocks per range group) is chosen per call from
the actual group sizes (max over all 16 sequences), so one compiled program is
shared by all cores; per-sequence raggedness is handled with runtime count
registers and the exp bias mask. Compiled programs are cached per layout.
"""
import sys
if '/opt/trn_rl_repo' not in sys.path:
    sys.path.insert(0, '/opt/trn_rl_repo')

import numpy as np

import concourse.bass as bass
import concourse.mybir as mybir
from concourse import bacc
from concourse.tile import TileContext
from concourse.masks import make_identity

# ---- problem constants (hardcoded per contract) ----
B, HQ, HKV, D, L = 16, 32, 8, 128, 4096
G = HQ // HKV                 # 4 query heads per kv head
SLOTS = B * (L + 1)           # 65552
SCALE = 0.08838834764831845
N_CORES = 8
SEQ_PER_CORE = B // N_CORES   # 2
ROW = HKV * D                 # 1024 elems = one k (or v) row
KVROW = 2 * ROW               # combined k|v row, 2048 fp16 elems = 4KB
BLK = 128                     # tokens per compute block
NEG_BIAS = -1.0e30

FP32 = mybir.dt.float32
FP16 = mybir.dt.float16
I32 = mybir.dt.int32
I16 = mybir.dt.int16

import os
ABLATE = os.environ.get('KERNEL_ABLATE', '')
GSZ = int(os.environ.get('KERNEL_GSZ', '512'))   # tokens per gather chunk
NQ = int(os.environ.get('KERNEL_NQ', '1'))       # SWDGE queues to spread over


# --------------------------------------------------------------------------
# program builder
# --------------------------------------------------------------------------

def build_program(nblks, reps=1):
    """nblks: tuple (nb_lo, nb_hi, nb_g2, nb_aux) block counts per group.
    Returns compiled Bacc program."""
    nb = list(nblks)
    NBLK = sum(nb)                       # compute blocks per sequence
    # chunks per group (each chunk = one dma_gather of up to GSZ tokens)
    nchunks = [(x * BLK + GSZ - 1) // GSZ for x in nb]
    CHUNKS = sum(nchunks)                # per (seq) count entries
    IDXC = CHUNKS * (GSZ // 16)          # idx cols per seq
    bias_cols = 2 * NBLK
    # global block j -> (global chunk index, sub-block within chunk)
    blockmap = []
    cbase = 0
    for gi in range(4):
        for bo in range(nb[gi]):
            blockmap.append((cbase + bo // (GSZ // BLK), bo % (GSZ // BLK)))
        cbase += nchunks[gi]

    nc = bacc.Bacc("TRN2", target_bir_lowering=False, debug=False,
                   num_devices=N_CORES, num_swdge_queues=NQ)
    kvc = nc.dram_tensor("kvc", [SLOTS, KVROW], FP16, kind="ExternalInput")
    kvaux = nc.dram_tensor("kvaux", [16, KVROW], FP16, kind="ExternalInput")
    qT = nc.dram_tensor("qT", [128, 2 * HQ], FP16, kind="ExternalInput")
    idx16 = nc.dram_tensor("idx16", [128, 2 * IDXC], I16, kind="ExternalInput")
    biasd = nc.dram_tensor("biasd", [128, bias_cols], FP32, kind="ExternalInput")
    cnts = nc.dram_tensor("cnts", [1, 2 * CHUNKS], I32, kind="ExternalInput")
    out = nc.dram_tensor("o", [128, 2 * HQ], FP32, kind="ExternalOutput")

    def group_src(gi):
        if gi == 0:
            return kvc[0:32768, :]
        if gi == 1:
            return kvc[32768:65536, :]
        if gi == 2:
            return kvc[65536:SLOTS, :]
        return kvaux[:, :]

    with TileContext(nc) as tc:
        with (
            tc.tile_pool(name="const", bufs=1) as cpool,
            tc.tile_pool(name="kv", bufs=3) as kvpool,
            tc.tile_pool(name="kt", bufs=2) as ktpool,
            tc.tile_pool(name="pt", bufs=3) as ptpool,
            tc.tile_pool(name="fin", bufs=1) as fpool,
            tc.tile_pool(name="ps_kt", bufs=2, space="PSUM") as ps_kt,
            tc.tile_pool(name="ps_st", bufs=3, space="PSUM") as ps_st,
            tc.tile_pool(name="ps_o", bufs=2, space="PSUM") as ps_o,
        ):
            ident = cpool.tile([128, 128], FP16)
            make_identity(nc, ident[:])
            ones_t = cpool.tile([128, 1], FP16)
            nc.vector.memset(ones_t[:], 1.0)
            qT_t = cpool.tile([128, 2 * HQ], FP16)
            nc.sync.dma_start(out=qT_t[:], in_=qT[:, :])
            idx_t = cpool.tile([128, 2 * IDXC], I16)
            nc.sync.dma_start(out=idx_t[:], in_=idx16[:, :])
            bias_t = cpool.tile([128, bias_cols], FP32)
            nc.sync.dma_start(out=bias_t[:], in_=biasd[:, :])
            cnt_t = cpool.tile([1, 2 * CHUNKS], I32)
            nc.sync.dma_start(out=cnt_t[:], in_=cnts[:, :])

            # per-seq accumulator bank: cols [0:32] = o^T, row0 cols [32:64]
            # = denom
            acc_tiles = [None, None]

            # No gather-tile pre-zeroing: the host pads the first 3 chunks of
            # seq 0 (the first use of each rotating buffer) to a full count
            # with dummy index 0, so every lane a matmul can read is either
            # this chunk's data or a previous chunk's - always finite fp16.

            for _rep in range(reps):
                for s in range(SEQ_PER_CORE):
                    acc_s = ps_o.tile([128, 2 * HQ], FP32, space="PSUM",
                                      tag="acc")
                    acc_tiles[s] = acc_s
                    # ---- gathers for this sequence ----
                    kvtiles = []
                    ci_global = 0
                    for gi in range(4):
                        src = group_src(gi)
                        for ci in range(nchunks[gi]):
                            col0 = (s * IDXC + ci_global * (GSZ // 16))
                            iap = idx_t[:, col0:col0 + GSZ // 16]
                            creg = nc.values_load(
                                cnt_t[0:1, s * CHUNKS + ci_global:
                                      s * CHUNKS + ci_global + 1],
                                engines=(mybir.EngineType.Pool,),
                                min_val=1, max_val=GSZ,
                                skip_runtime_bounds_check=True)
                            kv = kvpool.tile([128, (GSZ // BLK) * KVROW], FP16,
                                             tag="kv")
                            nc.gpsimd.dma_gather(
                                out_ap=kv[:].rearrange("p (j e) -> p j e",
                                                       e=KVROW),
                                in_ap=src, idxs_ap=iap,
                                num_idxs=GSZ, num_idxs_reg=creg,
                                elem_size=KVROW,
                                queue_num=ci_global % NQ)
                            kvtiles.append(kv)
                            ci_global += 1

                    # ---- compute blocks (software-pipelined emission:
                    # PE stream runs T(j), QK(j-1), PV(j-2) so cross-engine
                    # results (kT copy, exp) are ready a block early) ----
                    def stage_T(j):
                        ch, jj = blockmap[j]
                        kv = kvtiles[ch]
                        kT_ps = ps_kt.tile([128, ROW], FP16, space="PSUM",
                                           tag="kT_ps")
                        for h in range(HKV):
                            nc.tensor.transpose(
                                kT_ps[:, h*D:(h+1)*D],
                                kv[:, jj*KVROW + h*D: jj*KVROW + (h+1)*D],
                                ident[:])
                        kT_sb = ktpool.tile([128, ROW], FP16, tag="kt")
                        if j % 2 == 0:
                            nc.scalar.copy(kT_sb[:], kT_ps[:])
                        else:
                            nc.vector.tensor_copy(kT_sb[:], kT_ps[:])
                        return kT_sb

                    def stage_Q(j, kT_sb):
                        sT_ps = ps_st.tile([128, HQ], FP32, space="PSUM",
                                           tag="sT_ps")
                        for h in range(HKV):
                            nc.tensor.matmul(
                                sT_ps[:, h*G:(h+1)*G],
                                kT_sb[:, h*D:(h+1)*D],
                                qT_t[:, s*HQ + h*G: s*HQ + (h+1)*G],
                                start=True, stop=True)
                        pT = ptpool.tile([128, HQ], FP16, tag="pt")
                        nc.scalar.activation(
                            pT[:], sT_ps[:],
                            mybir.ActivationFunctionType.Exp,
                            bias=bias_t[:, s*NBLK + j: s*NBLK + j + 1],
                            scale=SCALE)
                        return pT

                    def stage_P(j, pT):
                        ch, jj = blockmap[j]
                        kv = kvtiles[ch]
                        voff = jj * KVROW + ROW
                        # one accumulation group per seq bank: started by the
                        # h=0 PV (covers all 128 partitions), denom rides along
                        nc.tensor.matmul(
                            acc_s[:, 0:G],
                            kv[:, voff: voff + D],
                            pT[:, 0:G],
                            start=(j == 0), stop=False)
                        nc.tensor.matmul(
                            acc_s[0:1, HQ:2*HQ],
                            ones_t[:], pT[:],
                            start=False, stop=False)
                        for h in range(1, HKV):
                            nc.tensor.matmul(
                                acc_s[:, h*G:(h+1)*G],
                                kv[:, voff + h*D: voff + (h+1)*D],
                                pT[:, h*G:(h+1)*G],
                                start=False,
                                stop=(j == NBLK - 1 and h == HKV - 1))

                    NB_RUN = NBLK if ABLATE != 'dmaonly' else 0
                    kTs, pTs = {}, {}
                    for jj in range(NB_RUN + 2):
                        if jj < NB_RUN:
                            kTs[jj] = stage_T(jj)
                        if 1 <= jj and jj - 1 < NB_RUN:
                            pTs[jj - 1] = stage_Q(jj - 1, kTs.pop(jj - 1))
                        if 2 <= jj and jj - 2 < NB_RUN:
                            stage_P(jj - 2, pTs.pop(jj - 2))

            # ---- normalize and write out ----
            o_sb = fpool.tile([128, 2 * HQ], FP32)
            if ABLATE == 'dmaonly':
                # keep a data dependency on the last gather tiles
                nc.vector.tensor_copy(o_sb[:, 0:1], kvtiles[-1][:, 0:1])
                nc.vector.tensor_copy(o_sb[:, 1:2], kvtiles[-2][:, 0:1])
            for s in range(SEQ_PER_CORE if ABLATE != 'dmaonly' else 0):
                acc_s = acc_tiles[s]
                rec = fpool.tile([1, HQ], FP32, tag=f"rec{s}")
                nc.vector.reciprocal(rec[:], acc_s[0:1, HQ:2*HQ])
                recb = fpool.tile([128, HQ], FP32, tag=f"recb{s}")
                nc.gpsimd.partition_broadcast(recb[:], rec[:])
                nc.vector.tensor_tensor(
                    out=o_sb[:, s*HQ:(s+1)*HQ], in0=acc_s[:, 0:HQ], in1=recb[:],
                    op=mybir.AluOpType.mult)
            nc.sync.dma_start(out=out[:, :], in_=o_sb[:])

    nc.compile()
    return nc


# --------------------------------------------------------------------------
# host-side input prep
# --------------------------------------------------------------------------

def balance_assignment(kv_len):
    """Greedy LPT: assign 16 seqs to 8 cores (2 each) minimizing max core
    token load (the kernel is gather-DMA-bound, so core time ~ sum kv_len).
    Returns perm with perm[2*c + s] = original sequence index."""
    order = np.argsort(-np.asarray(kv_len), kind="stable")
    loads = [0] * N_CORES
    slots = [[] for _ in range(N_CORES)]
    for b in order:
        c = min((c for c in range(N_CORES) if len(slots[c]) < 2),
                key=lambda c: loads[c])
        slots[c].append(int(b))
        loads[c] += int(kv_len[b])
    return np.array([b for c in range(N_CORES) for b in slots[c]], np.int64)


def prep_inputs(q, k, v, k_cache, v_cache, slot_mapping, kv_indices, kv_len):
    """Returns (nblks, in_maps, perm) — per-core input dicts."""
    q = np.asarray(q); k = np.asarray(k); v = np.asarray(v)
    k_cache = np.asarray(k_cache); v_cache = np.asarray(v_cache)
    slot_mapping = np.asarray(slot_mapping)
    kv_indices = np.asarray(kv_indices); kv_len = np.asarray(kv_len)
    perm = balance_assignment(kv_len)

    lut = np.full(SLOTS, -1, np.int32)
    lut[slot_mapping] = np.arange(16, dtype=np.int32)   # last write wins

    # per-seq group token lists
    groups = []   # groups[b] = [lo, hi, g2, aux] int arrays (rebased)
    for b in range(B):
        val = kv_indices[b, :int(kv_len[b])]
        a = lut[val]
        aux = a[a >= 0].astype(np.int64)
        rest = val[a < 0].astype(np.int64)
        lo = rest[rest < 32768]
        hi = rest[(rest >= 32768) & (rest < 65536)] - 32768
        g2 = rest[rest >= 65536] - 65536
        groups.append([lo, hi, g2, aux])

    nb = [max(1, max((len(groups[b][gi]) + BLK - 1) // BLK for b in range(B)))
          for gi in range(4)]
    nblks = tuple(nb)
    nchunks = [(x * BLK + GSZ - 1) // GSZ for x in nb]
    CHUNKS = sum(nchunks)
    NBLK = sum(nb)
    IDXC = CHUNKS * (GSZ // 16)

    # combined k|v rows, fp16
    kv16 = np.empty((SLOTS, KVROW), np.float16)
    kv16[:, :ROW] = k_cache.reshape(SLOTS, ROW)
    kv16[:, ROW:] = v_cache.reshape(SLOTS, ROW)
    kvaux = np.empty((16, KVROW), np.float16)
    kvaux[:, :ROW] = k.reshape(16, ROW)
    kvaux[:, ROW:] = v.reshape(16, ROW)

    in_maps = []
    for c in range(N_CORES):
        idxa = np.full((2, IDXC * 16), -1, np.int16).reshape(2, CHUNKS, GSZ)
        cnt = np.ones((2, CHUNKS), np.int32)
        biasa = np.full((128, 2 * NBLK), NEG_BIAS, np.float32)
        first_use = 3  # rotating gather bufs: first 3 chunks get full counts
        for s in range(SEQ_PER_CORE):
            b = int(perm[2 * c + s])
            ci_g = 0
            blk0 = 0
            for gi in range(4):
                toks = groups[b][gi]
                n = len(toks)
                for ci in range(nchunks[gi]):
                    part = toks[ci * GSZ:(ci + 1) * GSZ]
                    if len(part) == 0:
                        idxa[s, ci_g, 0] = 0     # dummy valid index, count 1
                        cnt[s, ci_g] = 1
                    else:
                        idxa[s, ci_g, :len(part)] = part.astype(np.int16)
                        cnt[s, ci_g] = len(part)
                    ci_g += 1
                # bias: valid lanes 0, masked lanes NEG_BIAS
                for bo in range(nb[gi]):
                    valid = min(max(n - bo * BLK, 0), BLK)
                    if valid > 0:
                        biasa[:valid, s * NBLK + blk0 + bo] = 0.0
                blk0 += nb[gi]
        # pad the first buffer-rotation chunks of seq 0 to full count with
        # dummy slot-0 indices (bias already masks those lanes) so the
        # device needs no gather-tile pre-zeroing
        for cg in range(first_use):
            pad = idxa[0, cg] < 0
            idxa[0, cg, pad] = 0
            cnt[0, cg] = GSZ
        # wrap idx to [16, x] then replicate to 128 partitions
        idx16 = np.concatenate(
            [np.tile(idxa[s].reshape(CHUNKS, GSZ // 16, 16)
                     .transpose(0, 2, 1).reshape(CHUNKS * 16, GSZ // 16)
                     .reshape(CHUNKS, 16, GSZ // 16)
                     .transpose(1, 0, 2).reshape(16, IDXC), (8, 1))
             for s in range(2)], axis=1)
        qTc = np.ascontiguousarray(
            q[perm[2*c:2*c+2]].transpose(2, 0, 1).reshape(128, 2 * HQ),
            np.float16)
        in_maps.append({
            "kvc": kv16,
            "kvaux": kvaux,
            "qT": qTc,
            "idx16": idx16,
            "biasd": biasa,
            "cnts": cnt.reshape(1, 2 * CHUNKS),
        })
    return nblks, in_maps, perm


def assemble_out(res, perm):
    """res: per-core {'o': [128, 64]} dicts -> full [B, HQ*D] output."""
    out = np.empty((B, HQ * D), np.float32)
    for c in range(N_CORES):
        o = res[c]["o"]                      # [128, 64] = [d, s*32+hq]
        for s in range(SEQ_PER_CORE):
            out[int(perm[2*c + s])] = o[:, s*HQ:(s+1)*HQ].T.reshape(HQ * D)
    return out


# --------------------------------------------------------------------------
# PJRT runner (replicated caches ship once)
# --------------------------------------------------------------------------

REPLICATED = ("kvc", "kvaux")


class BassRunner:
    def __init__(self, nc, n_cores, replicated=()):
        import jax
        from jax.sharding import Mesh, PartitionSpec, NamedSharding
        from jax.experimental.shard_map import shard_map
        from concourse.bass2jax import (_bass_exec_p, partition_id_tensor,
                                        install_neuronx_cc_hook)
        install_neuronx_cc_hook()
        self.jax = jax
        self.nc = nc
        self.n_cores = n_cores
        self.replicated = set(replicated)
        in_names, out_names, out_avals, zero_outs = [], [], [], []
        partition_name = (nc.partition_id_tensor.name
                          if nc.partition_id_tensor else None)
        for alloc in nc.m.functions[0].allocations:
            if not isinstance(alloc, mybir.MemoryLocationSet):
                continue
            name = alloc.memorylocations[0].name
            if alloc.kind == "ExternalInput":
                if name != partition_name:
                    in_names.append(name)
            elif alloc.kind == "ExternalOutput":
                shape = tuple(alloc.tensor_shape)
                dtype = mybir.dt.np(alloc.dtype)
                out_names.append(name)
                out_avals.append(jax.core.ShapedArray(shape, dtype))
                zero_outs.append(np.zeros(shape, dtype))
        self.in_names, self.out_names = in_names, out_names
        self.out_avals, self.zero_outs = out_avals, zero_outs
        n_params = len(in_names)
        all_in_names = list(in_names) + list(out_names)
        if partition_name is not None:
            all_in_names.append(partition_name)

        def _body(*args):
            operands = list(args)
            if partition_name is not None:
                operands.append(partition_id_tensor())
            outs = _bass_exec_p.bind(
                *operands, out_avals=tuple(out_avals),
                in_names=tuple(all_in_names), out_names=tuple(out_names),
                lowering_input_output_aliases=(),
                sim_require_finite=True, sim_require_nnan=True, nc=nc)
            return tuple(outs)

        devices = jax.devices()[:n_cores]
        self.mesh = Mesh(np.asarray(devices), ("core",))
        self.sharding = NamedSharding(self.mesh, PartitionSpec("core"))
        self.rep_sharding = NamedSharding(self.mesh, PartitionSpec())
        in_specs = tuple(
            PartitionSpec() if n in self.replicated else PartitionSpec("core")
            for n in in_names) + (PartitionSpec("core"),) * len(out_names)
        out_specs = (PartitionSpec("core"),) * len(out_names)
        self.fn = jax.jit(
            shard_map(_body, mesh=self.mesh, in_specs=in_specs,
                      out_specs=out_specs, check_rep=False),
            keep_unused=True)

    def put_inputs(self, in_maps):
        args = []
        for name in self.in_names:
            if name in self.replicated:
                args.append(self.jax.device_put(np.asarray(in_maps[0][name]),
                                                self.rep_sharding))
            else:
                concat = np.concatenate(
                    [np.asarray(m[name]) for m in in_maps], axis=0)
                args.append(self.jax.device_put(concat, self.sharding))
        for z in self.zero_outs:
            zz = np.zeros((self.n_cores * z.shape[0], *z.shape[1:]), z.dtype)
            args.append(self.jax.device_put(zz, self.sharding))
        return args

    def run(self, args):
        outs = self.fn(*args)
        self.jax.block_until_ready(outs)
        return outs

    def results(self, outs):
        return [
            {name: np.asarray(outs[i]).reshape(
                self.n_cores, *self.out_avals[i].shape)[c]
             for i, name in enumerate(self.out_names)}
            for c in range(self.n_cores)
        ]


_RUNNER_CACHE = {}


def get_runner(nblks, reps=1):
    key = (nblks, reps, ABLATE, GSZ, NQ)
    if key not in _RUNNER_CACHE:
        nc = build_program(nblks, reps=reps)
        _RUNNER_CACHE[key] = BassRunner(nc, N_CORES, replicated=REPLICATED)
    return _RUNNER_CACHE[key]


def kernel(**inputs) -> np.ndarray:
    nblks, in_maps, perm = prep_inputs(**inputs)
    runner = get_runner(nblks)
    args = runner.put_inputs(in_maps)
    res = runner.results(runner.run(args))
    return assemble_out(res, perm)
